# revision 1
# baseline (speedup 1.0000x reference)
"""Distributed 3-layer GCN surrogate model on 8 Trainium2 NeuronCores.

Strategy (per the node-partitioned data-parallel scheme):
  - nodes are sharded across the 8 cores (12500 dst nodes each); edges are
    colocated with their destination shard and sorted by destination.
  - the norm factorizes: out_d = dinv[d] * sum_e (dinv[src_e] * (h @ W)[src_e]),
    so each layer keeps a full replicated table  hws_l = dinv * (h_{l-1} @ W_l)
    built distributed + AllGather.
  - per-edge gathers run through the Q7 dma_gather engine (4 SWDGE queues in
    parallel); segment sums per 128-edge chunk are computed on the PE as
    one-hot Sel^T @ gathered matmuls accumulating 4 chunks per PSUM bank;
    partial sums are scattered to per-destination aggregation tables with
    dma_scatter_add (each (dst, src-bucket) written exactly once -> race free).
  - the source-node axis is split in 4 buckets of 25000 rows so gather indices
    fit int16; each bucket has its own zero-initialized aggregation table and
    the post-pass sums the 4 tables, applies dinv/bias/tanh, and produces the
    next layer's table slice (transform via PE transpose + matmul).
  - final global max-pool: per-core running max -> [128, 64] output; host
    reduces over cores/partitions and applies the tiny final linear layer.
"""

import numpy as np

P = 128
SELW = 32          # one-hot Sel width: max segments per 128-edge chunk
GCALL = 32         # chunks per dma_gather call (4096 edges)
BANKC = 512        # fp32 columns per PSUM bank / staging tile
NQ = 4             # SWDGE queues
FS = [16, 32, 64]  # aggregated feature width per layer (W1/W2/W3 out dims)


# ----------------------------------------------------------------- host plan

def _wrap16(lin, dtype=np.int16):
    """Q7 index layout: idx i at [i%16, i//16], replicated to all 8 core pairs."""
    n = len(lin)
    t = np.zeros((P, n // 16), dtype)
    idx = np.arange(n)
    for k in range(8):
        t[16 * k + idx % 16, idx // 16] = lin
    return t


def build_plan(x, W1, b1, W2, b2, W3, b3, edge_index, n_cores=8):
    n = x.shape[0]
    nloc = n // n_cores
    bsz = n // 4                     # src bucket size (int16-addressable)
    nt = (nloc + P - 1) // P         # node tiles per core
    nlocp = nt * P
    tail = nloc - (nt - 1) * P       # valid rows in the last tile
    sink = nlocp                     # scatter sink row
    aggrows = nlocp + P

    src = np.concatenate([edge_index[0], np.arange(n, dtype=np.int64)]).astype(np.int64)
    dst = np.concatenate([edge_index[1], np.arange(n, dtype=np.int64)]).astype(np.int64)
    deg = np.bincount(dst, minlength=n).astype(np.float64)
    dinv = (1.0 / np.sqrt(deg)).astype(np.float32)

    core_of = dst // nloc
    per = []   # per (core, bucket): dict with srcvals/segids/pids arrays
    nchunk_max = 0
    for c in range(n_cores):
        mc = core_of == c
        s_c, d_c = src[mc], dst[mc] - c * nloc
        row = []
        for r in range(4):
            mr = (s_c // bsz) == r
            s_r = (s_c[mr] - r * bsz).astype(np.int32)
            d_r = d_c[mr].astype(np.int32)
            order = np.argsort(d_r, kind="stable")
            s_r, d_r = s_r[order], d_r[order]
            cnts = np.bincount(d_r, minlength=nloc)
            dpres = np.nonzero(cnts)[0]
            cpres = cnts[dpres]
            assert cpres.max(initial=0) <= P, "segment exceeds one chunk"
            # greedy chunk packing: <=128 edges, <=SELW segments per chunk
            seg_chunk = np.empty(len(dpres), np.int32)
            seg_slot = np.empty(len(dpres), np.int32)
            seg_off = np.empty(len(dpres), np.int32)
            ck = fill = nseg = 0
            for i, cnt in enumerate(cpres):
                if fill + cnt > P or nseg == SELW:
                    ck += 1
                    fill = nseg = 0
                seg_chunk[i] = ck
                seg_slot[i] = nseg
                seg_off[i] = fill
                fill += cnt
                nseg += 1
            nchunk = ck + 1 if len(dpres) else 0
            srcvals = np.zeros((nchunk, P), np.int16)
            segids = np.full((nchunk, P), 63.0, np.float32)
            pids = np.full((nchunk, SELW), sink, np.int16)
            estart = np.concatenate([[0], np.cumsum(cpres)[:-1]])
            ech = np.repeat(seg_chunk, cpres)
            epos = np.repeat(seg_off, cpres) + (np.arange(len(s_r)) - np.repeat(estart, cpres))
            srcvals[ech, epos] = s_r
            segids[ech, epos] = np.repeat(seg_slot, cpres).astype(np.float32)
            pids[seg_chunk, seg_slot] = dpres
            row.append((srcvals, segids, pids))
            nchunk_max = max(nchunk_max, nchunk)
        per.append(row)

    NCHUNK = ((nchunk_max + GCALL - 1) // GCALL) * GCALL
    ncalls = 4 * (NCHUNK // GCALL)

    meta = dict(n=n, n_cores=n_cores, nloc=nloc, bsz=bsz, nt=nt, nlocp=nlocp,
                tail=tail, sink=sink, aggrows=aggrows, NCHUNK=NCHUNK, ncalls=ncalls)

    # per-layer scatter staging geometry
    meta["cps"] = [4 * (BANKC // F) for F in FS]             # chunks per staging
    meta["srows"] = [P * (BANKC // F) for F in FS]           # scatter rows per staging
    meta["nstage_b"] = [(NCHUNK + cps - 1) // cps for cps in meta["cps"]]

    ins = []
    for c in range(n_cores):
        d = {}
        xs = np.ascontiguousarray(x[c * nloc:(c + 1) * nloc].T)          # [128, nloc]
        d["xT"] = xs.astype(np.float32)
        dv = np.zeros(nlocp, np.float32)
        dv[:nloc] = dinv[c * nloc:(c + 1) * nloc]
        d["dinv_pt"] = np.ascontiguousarray(dv.reshape(nt, P).T)
        ng4 = (nt + 3) // 4
        dg = np.zeros((ng4, P, 4), np.float32)
        for t in range(nt):
            dg[t // 4, :, t % 4] = dv[t * P:(t + 1) * P]
        d["dinv_g"] = dg
        d["W1p"] = W1.astype(np.float32)                                  # [128,16]
        W2p = np.zeros((64, 32), np.float32); W2p[:16] = W2
        W3p = np.zeros((64, 64), np.float32); W3p[:32] = W3
        d["W2p"], d["W3p"] = W2p, W3p
        for li, (b, F) in enumerate(zip((b1, b2, b3), FS)):
            br = np.zeros((P, 64), np.float32); br[:, :F] = b[None, :]
            d[f"brep{li}"] = br
        d["iota32"] = np.tile(np.arange(SELW, dtype=np.float32), (P, 1))
        d["ident"] = np.eye(P, dtype=np.float32)

        gidx = np.zeros((ncalls, P, GCALL * P // 16), np.int16)
        segsel = np.zeros((ncalls, P, GCALL), np.float32)
        ci = 0
        for r in range(4):
            srcvals, segids, pids = per[c][r]
            nch = srcvals.shape[0]
            sv = np.zeros((NCHUNK, P), np.int16); sv[:nch] = srcvals
            sg = np.full((NCHUNK, P), 63.0, np.float32); sg[:nch] = segids
            pd = np.full((NCHUNK, SELW), sink, np.int16); pd[:nch] = pids
            per[c][r] = pd  # keep only pids for scatter phase
            for k in range(NCHUNK // GCALL):
                blk = sv[k * GCALL:(k + 1) * GCALL]        # [32, 128]
                lin = blk[(np.arange(GCALL * P) // P), (np.arange(GCALL * P) % P)]
                gidx[ci] = _wrap16(lin)
                segsel[ci] = sg[k * GCALL:(k + 1) * GCALL].T
                ci += 1
        d["gidx"] = gidx
        d["segsel"] = segsel

        for li, F in enumerate(FS):
            cps, srows, nst_b = meta["cps"][li], meta["srows"][li], meta["nstage_b"][li]
            sp = np.zeros((4 * nst_b, P, srows // 16), np.int16)
            si = 0
            for r in range(4):
                pd = per[c][r]
                for st in range(nst_b):
                    i = np.arange(srows)
                    cc = (i % P) // SELW + 4 * (i // P)
                    s = i % SELW
                    k = st * cps + cc
                    pid = np.where(k < NCHUNK, pd[np.minimum(k, NCHUNK - 1), s], sink)
                    sp[si] = _wrap16(pid.astype(np.int16))
                    si += 1
            d[f"spid{li}"] = sp
        for li in range(3):
            for r in range(4):
                d[f"aggz{li}_{r}"] = np.zeros((aggrows, 64), np.float32)
        ins.append(d)
    return ins, meta, dinv


# --------------------------------------------------------------- bass program

def _dma_gather_raw(nc, out_ap, in_ap, idxs_ap, num_idxs, elem_size, elem_step, queue_num):
    """bass dma_gather with the elem%256B assert relaxed (ucode only requires
    the table stride to be a multiple of 256B)."""
    import concourse.mybir as mybir
    g = nc.gpsimd
    g._assert_queue_num(queue_num)
    stride_bytes = elem_step * mybir.dt.size(in_ap.dtype)
    assert stride_bytes % 256 == 0
    _in_ap = g.lower_ap_dma(in_ap, for_custom_bir_dma=True)
    return g.add_instruction(
        mybir.InstDMAGatherAnt(
            name=g.bass.get_next_instruction_name(),
            ins=[*_in_ap, g.lower_ap(idxs_ap), g.lower_val_access(g.to_reg(num_idxs))],
            outs=[g.lower_ap(out_ap)],
            transpose=False, num_idxs=num_idxs, elem_size=elem_size,
            stride_bytes_256=stride_bytes // 256, gen_mode=0,
            single_packet=False, queue_num=queue_num,
            sbuf_tokens_per_rank=0, sbuf_free_dim_per_rank=0,
            sbuf_free_dim_pad_per_rank=0, sbuf_byte_offset=0,
        ))


def build_program(meta):
    import concourse.bass as bass
    import concourse.bacc as bacc
    import concourse.tile as tile
    import concourse.mybir as mybir
    f32, i16 = mybir.dt.float32, mybir.dt.int16

    n, n_cores = meta["n"], meta["n_cores"]
    nloc, bsz, nt, tail = meta["nloc"], meta["bsz"], meta["nt"], meta["tail"]
    aggrows, NCHUNK, ncalls = meta["aggrows"], meta["NCHUNK"], meta["ncalls"]

    nc = bacc.Bacc("TRN2", target_bir_lowering=False, debug=False,
                   num_devices=n_cores, num_swdge_queues=NQ)

    t_in = {}
    t_in["xT"] = nc.dram_tensor("xT", [P, nloc], f32, kind="ExternalInput")
    t_in["dinv_pt"] = nc.dram_tensor("dinv_pt", [P, nt], f32, kind="ExternalInput")
    ng4 = (nt + 3) // 4
    t_in["dinv_g"] = nc.dram_tensor("dinv_g", [ng4, P, 4], f32, kind="ExternalInput")
    t_in["W1p"] = nc.dram_tensor("W1p", [P, 16], f32, kind="ExternalInput")
    t_in["W2p"] = nc.dram_tensor("W2p", [64, 32], f32, kind="ExternalInput")
    t_in["W3p"] = nc.dram_tensor("W3p", [64, 64], f32, kind="ExternalInput")
    for li in range(3):
        t_in[f"brep{li}"] = nc.dram_tensor(f"brep{li}", [P, 64], f32, kind="ExternalInput")
    t_in["iota32"] = nc.dram_tensor("iota32", [P, SELW], f32, kind="ExternalInput")
    t_in["ident"] = nc.dram_tensor("ident", [P, P], f32, kind="ExternalInput")
    t_in["gidx"] = nc.dram_tensor("gidx", [ncalls, P, GCALL * P // 16], i16, kind="ExternalInput")
    t_in["segsel"] = nc.dram_tensor("segsel", [ncalls, P, GCALL], f32, kind="ExternalInput")
    for li in range(3):
        t_in[f"spid{li}"] = nc.dram_tensor(
            f"spid{li}", [4 * meta["nstage_b"][li], P, meta["srows"][li] // 16], i16,
            kind="ExternalInput")
        for r in range(4):
            t_in[f"aggz{li}_{r}"] = nc.dram_tensor(
                f"aggz{li}_{r}", [aggrows, 64], f32, kind="ExternalInput")
    pooled = nc.dram_tensor("pooled", [P, 64], f32, kind="ExternalOutput")

    tables = [nc.dram_tensor(f"hws{li}", [n, 64], f32, addr_space="Shared")
              for li in range(3)]
    bounces = [nc.dram_tensor(f"bounce{li}", [nloc, 64], f32) for li in range(3)]

    with tile.TileContext(nc) as tc:
        with (
            tc.tile_pool(name="const", bufs=1) as cpool,
            tc.tile_pool(name="gt", bufs=6) as gt_pool,
            tc.tile_pool(name="gi", bufs=6) as gi_pool,
            tc.tile_pool(name="seg", bufs=4) as seg_pool,
            tc.tile_pool(name="sel", bufs=4) as sel_pool,
            tc.tile_pool(name="sp", bufs=4) as sp_pool,
            tc.tile_pool(name="stage", bufs=4) as st_pool,
            tc.tile_pool(name="post", bufs=3) as post_pool,
            tc.tile_pool(name="hw", bufs=3) as hw_pool,
            tc.tile_pool(name="acc", bufs=1) as acc_pool,
            tc.tile_pool(name="pagg", bufs=3, space="PSUM") as pagg,
            tc.tile_pool(name="pmm", bufs=2, space="PSUM") as pmm,
        ):
            # ---- constants
            xT = cpool.tile([P, nloc], f32)
            nc.sync.dma_start(xT[:], t_in["xT"].ap())
            W1t = cpool.tile([P, 16], f32)
            nc.sync.dma_start(W1t[:], t_in["W1p"].ap())
            W2t = cpool.tile([64, 32], f32)
            nc.sync.dma_start(W2t[:], t_in["W2p"].ap())
            W3t = cpool.tile([64, 64], f32)
            nc.sync.dma_start(W3t[:], t_in["W3p"].ap())
            breps = []
            for li in range(3):
                bt = cpool.tile([P, 64], f32, tag=f"brep{li}")
                nc.sync.dma_start(bt[:], t_in[f"brep{li}"].ap())
                breps.append(bt)
            iota = cpool.tile([P, SELW], f32)
            nc.sync.dma_start(iota[:], t_in["iota32"].ap())
            ident = cpool.tile([P, P], f32)
            nc.sync.dma_start(ident[:], t_in["ident"].ap())
            dinv_cols = cpool.tile([P, nt], f32)   # dinv col per node tile
            nc.sync.dma_start(dinv_cols[:], t_in["dinv_pt"].ap())

            qrr = [0]

            def next_q():
                q = qrr[0] % NQ
                qrr[0] += 1
                return q

            # ---- phase A: hws0 = dinv * (x @ W1), distributed + AllGather
            for t in range(nt):
                m = P if t < nt - 1 else tail
                ps = pmm.tile([P, 64], f32, space="PSUM", tag="mm")
                nc.tensor.matmul(ps[:m, :16], xT[:, t * P:t * P + m], W1t[:],
                                 start=True, stop=True)
                hw = hw_pool.tile([P, 64], f32)
                nc.vector.tensor_scalar_mul(hw[:m, :16], ps[:m, :16],
                                            dinv_cols[:m, t:t + 1])
                nc.sync.dma_start(bounces[0].ap()[t * P:t * P + m, :16], hw[:m, :16])
            nc.gpsimd.collective_compute(
                "AllGather", mybir.AluOpType.bypass,
                replica_groups=[list(range(n_cores))],
                ins=[bounces[0].ap()], outs=[tables[0].ap()])

            # ---- layers
            acc = acc_pool.tile([P, 64], f32)
            for li in range(3):
                F = FS[li]
                cps, srows, nst_b = meta["cps"][li], meta["srows"][li], meta["nstage_b"][li]
                table = tables[li]
                # gather + Sel matmul + scatter partials, per src bucket
                cpg = cps // GCALL              # gather calls per staging
                for r in range(4):
                    agg_t = t_in[f"aggz{li}_{r}"]
                    gt = sel = None
                    for st in range(nst_b):
                        ps_st = pagg.tile([P, BANKC], f32, space="PSUM", tag="agg")
                        for kc in range(cpg):
                            k = st * cpg + kc
                            if k < NCHUNK // GCALL:
                                ci = r * (NCHUNK // GCALL) + k
                                it = gi_pool.tile([P, GCALL * P // 16], i16)
                                nc.sync.dma_start(it[:], t_in["gidx"].ap()[ci])
                                gt = gt_pool.tile([P, GCALL * F], f32)
                                _dma_gather_raw(
                                    nc, gt[:].rearrange("p (g f) -> p g f", f=F),
                                    table.ap()[r * bsz:(r + 1) * bsz, :F],
                                    it[:], GCALL * P, F, 64, next_q())
                                sg = seg_pool.tile([P, GCALL], f32)
                                nc.sync.dma_start(sg[:], t_in["segsel"].ap()[ci])
                                sel = sel_pool.tile([P, GCALL * SELW], f32)
                                for w in range(GCALL * SELW // BANKC):
                                    cw = BANKC // SELW
                                    nc.vector.tensor_tensor(
                                        out=sel[:, w * BANKC:(w + 1) * BANKC],
                                        in0=sg[:, w * cw:(w + 1) * cw]
                                            .rearrange("p (c o) -> p c o", o=1)
                                            .to_broadcast([P, cw, SELW]),
                                        in1=iota[:].rearrange("p (o s) -> p o s", o=1)
                                            .to_broadcast([P, cw, SELW]),
                                        op=mybir.AluOpType.is_equal)
                            for c in range(GCALL):
                                cc = kc * GCALL + c            # chunk in staging
                                pg, slot = cc % 4, cc // 4
                                nc.tensor.matmul(
                                    ps_st[pg * 32:(pg + 1) * 32, slot * F:(slot + 1) * F],
                                    sel[:, c * SELW:(c + 1) * SELW],
                                    gt[:, c * F:(c + 1) * F],
                                    start=True, stop=True,
                                    tile_position=(0, pg * 32))
                        st_tile = st_pool.tile([P, BANKC], f32)
                        nc.scalar.copy(out=st_tile[:], in_=ps_st[:])
                        spt = sp_pool.tile([P, srows // 16], i16)
                        nc.sync.dma_start(spt[:], t_in[f"spid{li}"].ap()[r * nst_b + st])
                        nc.gpsimd.dma_scatter_add(
                            out_ap=agg_t.ap()[:, :F],
                            in_ap=st_tile[:].rearrange("p (g f) -> p g f", f=F),
                            idxs_ap=spt[:], num_idxs=srows, num_idxs_reg=srows,
                            elem_size=F, elem_step=64,
                            single_packet=False, queue_num=next_q())

                # ---- post-pass over node tiles (groups of 4)
                for g in range(ng4):
                    t0 = g * 4
                    gt_n = min(4, nt - t0)
                    w = gt_n * 64
                    rows = t0 * P
                    a = []
                    for r in range(4):
                        at = post_pool.tile([P, 256], f32, tag=f"aggrd{r}")
                        nc.sync.dma_start(
                            at[:, :w],
                            t_in[f"aggz{li}_{r}"].ap()[rows:rows + gt_n * P, :]
                            .rearrange("(j p) f -> p j f", p=P))
                        a.append(at)
                    s01 = post_pool.tile([P, 256], f32, tag="s01")
                    nc.vector.tensor_tensor(out=s01[:, :w], in0=a[0][:, :w],
                                            in1=a[1][:, :w], op=mybir.AluOpType.add)
                    s23 = post_pool.tile([P, 256], f32, tag="s23")
                    nc.vector.tensor_tensor(out=s23[:, :w], in0=a[2][:, :w],
                                            in1=a[3][:, :w], op=mybir.AluOpType.add)
                    v = post_pool.tile([P, 256], f32, tag="v")
                    nc.vector.tensor_tensor(out=v[:, :w], in0=s01[:, :w],
                                            in1=s23[:, :w], op=mybir.AluOpType.add)
                    dg = post_pool.tile([P, 4], f32, tag="dg")
                    nc.sync.dma_start(dg[:], t_in["dinv_g"].ap()[g])
                    nc.vector.tensor_tensor(
                        out=v[:, :w], in0=v[:, :w],
                        in1=dg[:, :gt_n].rearrange("p (j o) -> p j o", o=1)
                            .to_broadcast([P, gt_n, 64]),
                        op=mybir.AluOpType.mult)
                    nc.vector.tensor_tensor(
                        out=v[:, :w], in0=v[:, :w],
                        in1=breps[li][:].rearrange("p (o f) -> p o f", o=1)
                            .to_broadcast([P, gt_n, 64]),
                        op=mybir.AluOpType.add)
                    h = post_pool.tile([P, 256], f32, tag="h")
                    nc.scalar.activation(h[:, :w], v[:, :w],
                                         mybir.ActivationFunctionType.Tanh)
                    for j in range(gt_n):
                        t = t0 + j
                        m = P if t < nt - 1 else tail
                        if li == 2:
                            hs = h[:m, j * 64:(j + 1) * 64]
                            if t == 0:
                                nc.vector.tensor_copy(out=acc[:m, :], in_=hs)
                            else:
                                nc.vector.tensor_tensor(out=acc[:m, :], in0=acc[:m, :],
                                                        in1=hs, op=mybir.AluOpType.max)
                        else:
                            psT = pmm.tile([P, P], f32, space="PSUM", tag="mmT")
                            nc.tensor.transpose(psT[:64, :], h[:, j * 64:(j + 1) * 64],
                                                ident[:])
                            hT = post_pool.tile([64, P], f32, tag="hT")
                            nc.scalar.copy(out=hT[:], in_=psT[:64, :])
                            Wn = W2t if li == 0 else W3t
                            Fn = FS[li + 1]
                            ps2 = pmm.tile([P, 64], f32, space="PSUM", tag="mm")
                            nc.tensor.matmul(ps2[:m, :Fn], hT[:, :m], Wn[:],
                                             start=True, stop=True)
                            hw = hw_pool.tile([P, 64], f32)
                            nc.vector.tensor_scalar_mul(hw[:m, :Fn], ps2[:m, :Fn],
                                                        dinv_cols[:m, t:t + 1])
                            nc.sync.dma_start(
                                bounces[li + 1].ap()[t * P:t * P + m, :Fn],
                                hw[:m, :Fn])
                if li < 2:
                    nc.gpsimd.collective_compute(
                        "AllGather", mybir.AluOpType.bypass,
                        replica_groups=[list(range(n_cores))],
                        ins=[bounces[li + 1].ap()], outs=[tables[li + 1].ap()])
            nc.sync.dma_start(pooled.ap(), acc[:])
    nc.compile()
    return nc


_CACHE = {}


def kernel(x, W1, b1, W2, b2, W3, b3, Wl, bl, edge_index):
    x = np.asarray(x); edge_index = np.asarray(edge_index)
    ins, meta, _ = build_plan(x, np.asarray(W1), np.asarray(b1), np.asarray(W2),
                              np.asarray(b2), np.asarray(W3), np.asarray(b3),
                              edge_index, n_cores=8)
    key = (x.shape, edge_index.shape, meta["NCHUNK"])
    if key not in _CACHE:
        _CACHE[key] = build_program(meta)
    nc = _CACHE[key]
    from concourse.bass_utils import run_bass_kernel_spmd
    res = run_bass_kernel_spmd(nc, ins, core_ids=list(range(8)))
    pool = np.stack([res.results[c]["pooled"] for c in range(8)])  # [8,128,64]
    pooled = pool.max(axis=(0, 1))[:64].astype(np.float32)          # [64]
    out = pooled[None, :] @ np.asarray(Wl, np.float32) + np.asarray(bl, np.float32)
    return out.astype(np.float32)



# revision 2
# speedup vs baseline: 1.2419x; 1.2419x over previous
"""Distributed 3-layer GCN on 8 Trainium2 NeuronCores — v2.

vs v1: no dma_scatter_add, no DRAM aggregation tables. Edges are sorted by
destination and chunked within 128-dst aligned windows; each chunk's
segment-sum matmul accumulates directly into the window's PSUM slot
(start/stop flags). Tables are bf16 ([n,128] rows, 256B stride), gathers
are bf16, and the 4 src buckets map to the 4 SWDGE queues so Q7
descriptor generation runs on all four core pairs in parallel. The
schedule is padded per (window,bucket) to the max chunk count over cores
so one SPMD program serves all 8 cores; gather indices and segment ids
are SBUF-resident per layer.
"""

import numpy as np

P = 128
GCALL = 32           # max chunks per gather call (4096 idxs)
BANKC = 512          # fp32 cols per PSUM bank
FS = [16, 32, 64]    # layer output widths
GE = [32, 32, 64]    # gathered bf16 elems per layer (>=F; 64B/64B/128B)
NB = 4               # src buckets (int16-addressable tables)


def _wrap16(lin, dtype=np.int16):
    n = len(lin)
    t = np.zeros((P, (n + 15) // 16), dtype)
    idx = np.arange(n)
    for k in range(8):
        t[16 * k + idx % 16, idx // 16] = lin
    return t


def build_plan(x, W1, b1, W2, b2, W3, b3, edge_index, n_cores=8):
    import ml_dtypes
    bf16 = ml_dtypes.bfloat16
    n = x.shape[0]
    nloc = n // n_cores
    bsz = n // NB
    nw = (nloc + P - 1) // P

    src = np.concatenate([edge_index[0], np.arange(n, dtype=np.int64)])
    dst = np.concatenate([edge_index[1], np.arange(n, dtype=np.int64)])
    deg = np.bincount(dst, minlength=n).astype(np.float64)
    dinv = (1.0 / np.sqrt(deg)).astype(np.float32)
    core_of = dst // nloc

    # per-core chunk decomposition: (window, bucket) cells, <=128 edges/chunk
    percore = []
    for c in range(n_cores):
        mc = core_of == c
        s_c = src[mc]
        d_c = (dst[mc] - c * nloc).astype(np.int64)
        r_c = s_c // bsz
        srel = (s_c - r_c * bsz).astype(np.int32)
        w_c = d_c // P
        order = np.lexsort((d_c, r_c, w_c))
        percore.append((srel[order], d_c[order], r_c[order], w_c[order]))

    # uniform chunk counts per (window, bucket): max over cores
    cnt = np.zeros((nw, NB), np.int64)
    for (s_o, d_o, r_o, w_o) in percore:
        e = np.zeros((nw, NB), np.int64)
        np.add.at(e, (w_o, r_o), 1)
        cnt = np.maximum(cnt, (e + P - 1) // P)
    cnt = np.maximum(cnt, 1)          # every cell gets >=1 chunk slot
    tot = int(cnt.sum())              # chunks per core per layer (uniform)

    # global chunk id in (w, r, k) order
    cell_off = np.zeros((nw, NB), np.int64)
    cell_off.flat[1:] = np.cumsum(cnt.flat)[:-1]

    meta = dict(n=n, n_cores=n_cores, nloc=nloc, bsz=bsz, nw=nw,
                cnt=cnt, cell_off=cell_off, tot=tot)

    ins = []
    for c in range(n_cores):
        s_o, d_o, r_o, w_o = percore[c]
        ne = len(s_o)
        # position within (w, r) cell
        grp = w_o * NB + r_o
        newg = np.empty(ne, bool)
        newg[0] = True
        newg[1:] = grp[1:] != grp[:-1]
        gstarts = np.flatnonzero(newg)
        pos = np.arange(ne) - np.repeat(gstarts, np.diff(np.append(gstarts, ne)))
        chunk = cell_off[w_o, r_o] + pos // P
        slot = pos % P

        srcv = np.zeros((tot, P), np.int16)
        segv = np.full((tot, P), 255.0, np.float32)
        srcv[chunk, slot] = s_o.astype(np.int16)
        segv[chunk, slot] = (d_o % P).astype(np.float32)

        d = {}
        d["xT"] = np.ascontiguousarray(
            x[c * nloc:(c + 1) * nloc].T).astype(np.float32)
        dv = np.zeros(nw * P, np.float32)
        dv[:nloc] = dinv[c * nloc:(c + 1) * nloc]
        d["dinv_w"] = np.ascontiguousarray(dv.reshape(nw, P).T)
        d["W1p"] = np.asarray(W1, np.float32)
        W2p = np.zeros((64, 32), np.float32); W2p[:16] = W2
        W3p = np.zeros((64, 64), np.float32); W3p[:32] = W3
        d["W2p"], d["W3p"] = W2p, W3p
        for li, (b, F) in enumerate(zip((b1, b2, b3), FS)):
            br = np.zeros((P, 64), np.float32); br[:, :F] = np.asarray(b)[None, :]
            d[f"brep{li}"] = br
        d["iota128"] = np.tile(np.arange(P, dtype=np.float32), (P, 1)).astype(bf16)
        d["ident"] = np.eye(P, dtype=np.float32)
        d["z16"] = np.zeros((P, 16), bf16)
        d["gseg"] = np.ascontiguousarray(segv.T).astype(bf16)   # [P, tot]

        # gather idx flat layout in CALL order (bucket-major runs of <=32
        # chunks within each bucket, windows ascending)
        calls = plan_calls(cnt, cell_off)
        gidx = np.zeros((P, tot * P // 16), np.int16)
        off = 0
        for (chunks, r) in calls:
            lin = srcv[chunks].reshape(-1)
            w = _wrap16(lin)
            gidx[:, off:off + w.shape[1]] = w
            off += w.shape[1]
        d["gidx"] = gidx
        ins.append(d)
    return ins, meta, dinv


def plan_calls(cnt, cell_off):
    """Call schedule: per bucket, runs of <=GCALL chunks over ascending
    windows. Returns list of (chunk_id_array, bucket)."""
    nw = cnt.shape[0]
    calls = []
    for r in range(NB):
        ids = np.concatenate([cell_off[w, r] + np.arange(cnt[w, r])
                              for w in range(nw)])
        for i in range(0, len(ids), GCALL):
            calls.append((ids[i:i + GCALL], r))
    return calls



def dma_gather_raw(nc, out_ap, in_ap, idxs_ap, num_idxs, elem_size, elem_step,
                   queue_num, single_packet=False):
    """bass dma_gather with the elem%256B assert relaxed (ucode only requires
    the table stride to be a multiple of 256B)."""
    import concourse.mybir as mybir
    g = nc.gpsimd
    g._assert_queue_num(queue_num)
    stride_bytes = elem_step * mybir.dt.size(in_ap.dtype)
    assert stride_bytes % 256 == 0
    _in_ap = g.lower_ap_dma(in_ap, for_custom_bir_dma=True)
    return g.add_instruction(
        mybir.InstDMAGatherAnt(
            name=g.bass.get_next_instruction_name(),
            ins=[*_in_ap, g.lower_ap(idxs_ap), g.lower_val_access(g.to_reg(num_idxs))],
            outs=[g.lower_ap(out_ap)],
            transpose=False, num_idxs=num_idxs, elem_size=elem_size,
            stride_bytes_256=stride_bytes // 256, gen_mode=0,
            single_packet=single_packet, queue_num=queue_num,
            sbuf_tokens_per_rank=0, sbuf_free_dim_per_rank=0,
            sbuf_free_dim_pad_per_rank=0, sbuf_byte_offset=0,
        ))

def build_program(meta):
    import concourse.bass as bass
    import concourse.bacc as bacc
    import concourse.tile as tile
    import concourse.mybir as mybir
    f32, i16, bf16 = mybir.dt.float32, mybir.dt.int16, mybir.dt.bfloat16

    n, n_cores = meta["n"], meta["n_cores"]
    nloc, nw, tot = meta["nloc"], meta["nw"], meta["tot"]
    cnt, cell_off = meta["cnt"], meta["cell_off"]
    calls = plan_calls(cnt, cell_off)
    # chunk -> (call index, position in call); calls are contiguous id runs
    chunk_call = np.zeros(tot, np.int64)
    chunk_cpos = np.zeros(tot, np.int64)
    for ci, (ids, r) in enumerate(calls):
        chunk_call[ids] = ci
        chunk_cpos[ids] = np.arange(len(ids))
    call_off = np.cumsum([0] + [len(ids) for ids, _ in calls])  # chunk cols

    nc = bacc.Bacc("TRN2", target_bir_lowering=False, debug=False,
                   num_devices=n_cores, num_swdge_queues=NB)

    t_in = {}
    t_in["xT"] = nc.dram_tensor("xT", [P, nloc], f32, kind="ExternalInput")
    t_in["dinv_w"] = nc.dram_tensor("dinv_w", [P, nw], f32, kind="ExternalInput")
    t_in["W1p"] = nc.dram_tensor("W1p", [P, 16], f32, kind="ExternalInput")
    t_in["W2p"] = nc.dram_tensor("W2p", [64, 32], f32, kind="ExternalInput")
    t_in["W3p"] = nc.dram_tensor("W3p", [64, 64], f32, kind="ExternalInput")
    for li in range(3):
        t_in[f"brep{li}"] = nc.dram_tensor(f"brep{li}", [P, 64], f32,
                                           kind="ExternalInput")
    t_in["iota128"] = nc.dram_tensor("iota128", [P, P], bf16, kind="ExternalInput")
    t_in["ident"] = nc.dram_tensor("ident", [P, P], f32, kind="ExternalInput")
    t_in["z16"] = nc.dram_tensor("z16", [P, 16], bf16, kind="ExternalInput")
    t_in["gidx"] = nc.dram_tensor("gidx", [P, tot * P // 16], i16,
                                  kind="ExternalInput")
    t_in["gseg"] = nc.dram_tensor("gseg", [P, tot], bf16, kind="ExternalInput")
    pooled = nc.dram_tensor("pooled", [P, 64], f32, kind="ExternalOutput")

    tables = [nc.dram_tensor(f"hws{li}", [n, 128], bf16, addr_space="Shared")
              for li in range(3)]
    bounces = [nc.dram_tensor(f"bounce{li}", [nloc, 128], bf16)
               for li in range(3)]

    with tile.TileContext(nc) as tc:
        with (
            tc.tile_pool(name="const", bufs=1) as cpool,
            tc.tile_pool(name="gt", bufs=8) as gt_pool,
            tc.tile_pool(name="sel", bufs=8) as sel_pool,
            tc.tile_pool(name="stage", bufs=2) as st_pool,
            tc.tile_pool(name="post", bufs=2) as post_pool,
            tc.tile_pool(name="hw", bufs=4) as hw_pool,
            tc.tile_pool(name="acc", bufs=1) as acc_pool,
            tc.tile_pool(name="pagg", bufs=2, space="PSUM") as pagg,
            tc.tile_pool(name="pmm", bufs=3, space="PSUM") as pmm,
        ):
            W1t = cpool.tile([P, 16], f32)
            nc.sync.dma_start(W1t[:], t_in["W1p"].ap())
            W2t = cpool.tile([64, 32], f32)
            nc.sync.dma_start(W2t[:], t_in["W2p"].ap())
            W3t = cpool.tile([64, 64], f32)
            nc.sync.dma_start(W3t[:], t_in["W3p"].ap())
            breps = []
            for li in range(3):
                bt = cpool.tile([P, 64], f32, tag=f"brep{li}")
                nc.sync.dma_start(bt[:], t_in[f"brep{li}"].ap())
                breps.append(bt)
            iota = cpool.tile([P, P], bf16)
            nc.sync.dma_start(iota[:], t_in["iota128"].ap())
            ident = cpool.tile([P, P], f32)
            nc.sync.dma_start(ident[:], t_in["ident"].ap())
            z16 = cpool.tile([P, 16], bf16)
            nc.sync.dma_start(z16[:], t_in["z16"].ap())
            dinv_w = cpool.tile([P, nw], f32)
            nc.sync.dma_start(dinv_w[:], t_in["dinv_w"].ap())
            gidx = cpool.tile([P, tot * P // 16], i16)
            nc.sync.dma_start(gidx[:], t_in["gidx"].ap())
            gseg = cpool.tile([P, tot], bf16)
            nc.sync.dma_start(gseg[:], t_in["gseg"].ap())

            # ---- phase A: table0 rows = dinv * (x @ W1), bf16, then AG
            with tc.tile_pool(name="xa", bufs=3) as xa_pool:
                for t in range(nw):
                    m = min(P, nloc - t * P)
                    xT = xa_pool.tile([P, P], f32, tag="xt")
                    nc.sync.dma_start(xT[:, :m], t_in["xT"].ap()[:, t * P:t * P + m])
                    ps = pmm.tile([P, 64], f32, space="PSUM", tag="mm")
                    nc.tensor.matmul(ps[:m, :16], xT[:, :m],
                                     W1t[:], start=True, stop=True)
                    hw = hw_pool.tile([P, 32], bf16)
                    nc.vector.tensor_scalar_mul(hw[:m, :16], ps[:m, :16],
                                                dinv_w[:m, t:t + 1])
                    nc.vector.tensor_copy(out=hw[:m, 16:32], in_=z16[:m])
                    nc.sync.dma_start(bounces[0].ap()[t * P:t * P + m, :32],
                                      hw[:m, :])
            nc.gpsimd.collective_compute(
                "AllGather", mybir.AluOpType.bypass,
                replica_groups=[list(range(n_cores))],
                ins=[bounces[0].ap()], outs=[tables[0].ap()])

            acc = acc_pool.tile([P, 64], f32)

            for li in range(3):
                F, GEl = FS[li], GE[li]
                wps = BANKC // F
                nst = (nw + wps - 1) // wps
                table = tables[li]
                gt_tiles = {}      # call idx -> tile
                emitted = set()

                def emit_call(ci):
                    ids, r = calls[ci]
                    ncc = len(ids)
                    it_ap = gidx[:, call_off[ci] * 8:(call_off[ci] + ncc) * 8]
                    gt = gt_pool.tile([P, GCALL * GEl], bf16, tag=f"gt{li}")
                    dma_gather_raw(
                        nc, gt[:, :ncc * GEl].rearrange("p (g f) -> p g f",
                                                        f=GEl),
                        table.ap()[r * meta["bsz"]:(r + 1) * meta["bsz"], :GEl],
                        it_ap, ncc * P, GEl, 128, r)
                    gt_tiles[ci] = gt
                    emitted.add(ci)

                for st in range(nst):
                    wlo = st * wps
                    wcnt = min(wps, nw - wlo)
                    ps_st = pagg.tile([P, BANKC], f32, space="PSUM", tag="agg")
                    # gathers covering this staging's chunks, emitted
                    # round-robin across buckets so window-major consumption
                    # can retire buffers (8-buf pool) without WAR cycles
                    need = set()
                    for w in range(wlo, wlo + wcnt):
                        for r in range(NB):
                            for k in range(cnt[w, r]):
                                need.add(int(chunk_call[cell_off[w, r] + k]))
                    by_r = {}
                    for ci in sorted(need):
                        if ci not in emitted:
                            by_r.setdefault(calls[ci][1], []).append(ci)
                    maxlen = max((len(v) for v in by_r.values()), default=0)
                    for k in range(maxlen):
                        for r in sorted(by_r):
                            if k < len(by_r[r]):
                                emit_call(by_r[r][k])
                    # matmuls, window-major
                    for j in range(wcnt):
                        w = wlo + j
                        ids = [int(cell_off[w, r] + k)
                               for r in range(NB) for k in range(cnt[w, r])]
                        # batched sel build for this window's chunks
                        nk = len(ids)
                        sel = sel_pool.tile([P, nk * P], bf16, tag="sel")
                        # chunk ids are contiguous per (w,r) cell; cells of
                        # one window are contiguous in (w,r,k) order
                        g0 = ids[0]
                        nc.vector.tensor_tensor(
                            out=sel[:].rearrange("p (c s) -> p c s", s=P),
                            in0=gseg[:, g0:g0 + nk]
                                .rearrange("p (c o) -> p c o", o=1)
                                .to_broadcast([P, nk, P]),
                            in1=iota[:].rearrange("p (o s) -> p o s", o=1)
                                .to_broadcast([P, nk, P]),
                            op=mybir.AluOpType.is_equal)
                        for q, g in enumerate(ids):
                            ci = int(chunk_call[g])
                            cp = int(chunk_cpos[g])
                            gt = gt_tiles[ci]
                            nc.tensor.matmul(
                                ps_st[:, (j * F):(j * F) + F],
                                sel[:, q * P:(q + 1) * P],
                                gt[:, cp * GEl:cp * GEl + F],
                                start=(q == 0), stop=(q == nk - 1))
                    # drain + post
                    stg = st_pool.tile([P, BANKC], f32, tag="stg")
                    nc.scalar.copy(out=stg[:, :wcnt * F], in_=ps_st[:, :wcnt * F])
                    v = post_pool.tile([P, BANKC], f32, tag="v")
                    nc.vector.tensor_tensor(
                        out=v[:, :wcnt * F].rearrange("p (j f) -> p j f", f=F),
                        in0=stg[:, :wcnt * F].rearrange("p (j f) -> p j f", f=F),
                        in1=dinv_w[:, wlo:wlo + wcnt]
                            .rearrange("p (j o) -> p j o", o=1)
                            .to_broadcast([P, wcnt, F]),
                        op=mybir.AluOpType.mult)
                    nc.vector.tensor_tensor(
                        out=v[:, :wcnt * F].rearrange("p (j f) -> p j f", f=F),
                        in0=v[:, :wcnt * F].rearrange("p (j f) -> p j f", f=F),
                        in1=breps[li][:, :F].rearrange("p (o f) -> p o f", o=1)
                            .to_broadcast([P, wcnt, F]),
                        op=mybir.AluOpType.add)
                    h = post_pool.tile([P, BANKC], f32, tag="h")
                    nc.scalar.activation(h[:, :wcnt * F], v[:, :wcnt * F],
                                         mybir.ActivationFunctionType.Tanh)
                    for j in range(wcnt):
                        w = wlo + j
                        m = min(P, nloc - w * P)
                        if m <= 0:
                            continue
                        if li == 2:
                            hs = h[:m, j * F:(j + 1) * F]
                            if w == 0:
                                nc.vector.tensor_copy(out=acc[:m, :], in_=hs)
                            else:
                                nc.vector.tensor_tensor(
                                    out=acc[:m, :], in0=acc[:m, :], in1=hs,
                                    op=mybir.AluOpType.max)
                        else:
                            Fn = FS[li + 1]
                            psT = pmm.tile([P, P], f32, space="PSUM", tag="mmT")
                            nc.tensor.transpose(psT[:F, :],
                                                h[:, j * F:(j + 1) * F],
                                                ident[:])
                            hT = post_pool.tile([64, P], f32, tag="hT")
                            nc.scalar.copy(out=hT[:], in_=psT[:64, :])
                            Wn = W2t if li == 0 else W3t
                            ps2 = pmm.tile([P, 64], f32, space="PSUM", tag="mm")
                            nc.tensor.matmul(ps2[:m, :Fn], hT[:, :m], Wn[:],
                                             start=True, stop=True)
                            hwt = hw_pool.tile([P, 64], bf16, tag="hwb")
                            nc.vector.tensor_scalar_mul(
                                hwt[:m, :Fn], ps2[:m, :Fn], dinv_w[:m, w:w + 1])
                            nc.sync.dma_start(
                                bounces[li + 1].ap()[w * P:w * P + m, :Fn],
                                hwt[:m, :Fn])
                if li < 2:
                    nc.gpsimd.collective_compute(
                        "AllGather", mybir.AluOpType.bypass,
                        replica_groups=[list(range(n_cores))],
                        ins=[bounces[li + 1].ap()], outs=[tables[li + 1].ap()])
            nc.sync.dma_start(pooled.ap(), acc[:])
    nc.compile()
    return nc


_CACHE = {}


def kernel(x, W1, b1, W2, b2, W3, b3, Wl, bl, edge_index):
    x = np.asarray(x); edge_index = np.asarray(edge_index)
    ins, meta, _ = build_plan(x, np.asarray(W1), np.asarray(b1), np.asarray(W2),
                              np.asarray(b2), np.asarray(W3), np.asarray(b3),
                              edge_index, n_cores=8)
    key = (x.shape, edge_index.shape, meta["tot"])
    if key not in _CACHE:
        _CACHE[key] = build_program(meta)
    nc = _CACHE[key]
    from concourse.bass_utils import run_bass_kernel_spmd
    res = run_bass_kernel_spmd(nc, ins, core_ids=list(range(8)))
    pool = np.stack([res.results[c]["pooled"] for c in range(8)])
    pooled = pool.max(axis=(0, 1))[:64].astype(np.float32)
    out = pooled[None, :] @ np.asarray(Wl, np.float32) + np.asarray(bl, np.float32)
    return out.astype(np.float32)


# revision 3
# speedup vs baseline: 1.3351x; 1.0751x over previous
"""Distributed 3-layer GCN on 8 Trainium2 NeuronCores — v2.

vs v1: no dma_scatter_add, no DRAM aggregation tables. Edges are sorted by
destination and chunked within 128-dst aligned windows; each chunk's
segment-sum matmul accumulates directly into the window's PSUM slot
(start/stop flags). Tables are bf16 ([n,128] rows, 256B stride), gathers
are bf16, and the 4 src buckets map to the 4 SWDGE queues so Q7
descriptor generation runs on all four core pairs in parallel. The
schedule is padded per (window,bucket) to the max chunk count over cores
so one SPMD program serves all 8 cores; gather indices and segment ids
are SBUF-resident per layer.
"""

import numpy as np

P = 128
GCALL = 32           # max chunks per gather call (4096 idxs)
BANKC = 512          # fp32 cols per PSUM bank
FS = [16, 32, 64]    # layer output widths
GE = [32, 32, 64]    # gathered bf16 elems per layer (>=F; 64B/64B/128B)
NB = 4               # src buckets (int16-addressable tables)


def _wrap16(lin, dtype=np.int16):
    n = len(lin)
    t = np.zeros((P, (n + 15) // 16), dtype)
    idx = np.arange(n)
    for k in range(8):
        t[16 * k + idx % 16, idx // 16] = lin
    return t


def build_plan(x, W1, b1, W2, b2, W3, b3, edge_index, n_cores=8):
    import ml_dtypes
    bf16 = ml_dtypes.bfloat16
    n = x.shape[0]
    nloc = n // n_cores
    bsz = n // NB
    nw = (nloc + P - 1) // P

    # self-loops are handled in the post-pass (local rows, no gather);
    # degree still counts them (PyG default adds them before normalizing)
    src = np.asarray(edge_index[0])
    dst = np.asarray(edge_index[1])
    deg = (np.bincount(dst, minlength=n) + 1).astype(np.float64)
    dinv = (1.0 / np.sqrt(deg)).astype(np.float32)
    core_of = dst // nloc

    # per-core chunk decomposition: (window, bucket) cells, <=128 edges/chunk
    percore = []
    for c in range(n_cores):
        mc = core_of == c
        s_c = src[mc]
        d_c = (dst[mc] - c * nloc).astype(np.int64)
        r_c = s_c // bsz
        srel = (s_c - r_c * bsz).astype(np.int32)
        w_c = d_c // P
        order = np.lexsort((d_c, r_c, w_c))
        percore.append((srel[order], d_c[order], r_c[order], w_c[order]))

    # uniform chunk counts per (window, bucket): max over cores
    cnt = np.zeros((nw, NB), np.int64)
    for (s_o, d_o, r_o, w_o) in percore:
        e = np.zeros((nw, NB), np.int64)
        np.add.at(e, (w_o, r_o), 1)
        cnt = np.maximum(cnt, (e + P - 1) // P)
    cnt = np.maximum(cnt, 1)          # every cell gets >=1 chunk slot
    tot = int(cnt.sum())              # chunks per core per layer (uniform)

    # global chunk id in (w, r, k) order
    cell_off = np.zeros((nw, NB), np.int64)
    cell_off.flat[1:] = np.cumsum(cnt.flat)[:-1]

    meta = dict(n=n, n_cores=n_cores, nloc=nloc, bsz=bsz, nw=nw,
                cnt=cnt, cell_off=cell_off, tot=tot)

    ins = []
    for c in range(n_cores):
        s_o, d_o, r_o, w_o = percore[c]
        ne = len(s_o)
        # position within (w, r) cell
        grp = w_o * NB + r_o
        newg = np.empty(ne, bool)
        newg[0] = True
        newg[1:] = grp[1:] != grp[:-1]
        gstarts = np.flatnonzero(newg)
        pos = np.arange(ne) - np.repeat(gstarts, np.diff(np.append(gstarts, ne)))
        chunk = cell_off[w_o, r_o] + pos // P
        slot = pos % P

        srcv = np.zeros((tot, P), np.int16)
        segv = np.full((tot, P), 255.0, np.float32)
        srcv[chunk, slot] = s_o.astype(np.int16)
        segv[chunk, slot] = (d_o % P).astype(np.float32)

        d = {}
        d["xT"] = np.ascontiguousarray(
            x[c * nloc:(c + 1) * nloc].T).astype(np.float32)
        dv = np.zeros(nw * P, np.float32)
        dv[:nloc] = dinv[c * nloc:(c + 1) * nloc]
        d["dinv_w"] = np.ascontiguousarray(dv.reshape(nw, P).T)
        d["W1p"] = np.asarray(W1, np.float32)
        W2p = np.zeros((64, 32), np.float32); W2p[:16] = W2
        W3p = np.zeros((64, 64), np.float32); W3p[:32] = W3
        d["W2p"], d["W3p"] = W2p, W3p
        for li, (b, F) in enumerate(zip((b1, b2, b3), FS)):
            br = np.zeros((P, 64), np.float32); br[:, :F] = np.asarray(b)[None, :]
            d[f"brep{li}"] = br
        d["iota128"] = np.tile(np.arange(P, dtype=np.float32), (P, 1)).astype(bf16)
        d["ident"] = np.eye(P, dtype=np.float32)
        d["z16"] = np.zeros((P, 16), bf16)
        d["gseg"] = np.ascontiguousarray(segv.T).astype(bf16)   # [P, tot]

        # gather idx flat layout in CALL order (bucket-major runs of <=32
        # chunks within each bucket, windows ascending)
        calls = plan_calls(cnt, cell_off)
        gidx = np.zeros((P, tot * P // 16), np.int16)
        off = 0
        for (chunks, r) in calls:
            lin = srcv[chunks].reshape(-1)
            w = _wrap16(lin)
            gidx[:, off:off + w.shape[1]] = w
            off += w.shape[1]
        d["gidx"] = gidx
        ins.append(d)
    return ins, meta, dinv


def plan_calls(cnt, cell_off):
    """Call schedule: per bucket, runs of <=GCALL chunks over ascending
    windows. Returns list of (chunk_id_array, bucket)."""
    nw = cnt.shape[0]
    calls = []
    for r in range(NB):
        ids = np.concatenate([cell_off[w, r] + np.arange(cnt[w, r])
                              for w in range(nw)])
        for i in range(0, len(ids), GCALL):
            calls.append((ids[i:i + GCALL], r))
    return calls



def dma_gather_raw(nc, out_ap, in_ap, idxs_ap, num_idxs, elem_size, elem_step,
                   queue_num, single_packet=False):
    """bass dma_gather with the elem%256B assert relaxed (ucode only requires
    the table stride to be a multiple of 256B)."""
    import concourse.mybir as mybir
    g = nc.gpsimd
    g._assert_queue_num(queue_num)
    stride_bytes = elem_step * mybir.dt.size(in_ap.dtype)
    assert stride_bytes % 256 == 0
    _in_ap = g.lower_ap_dma(in_ap, for_custom_bir_dma=True)
    return g.add_instruction(
        mybir.InstDMAGatherAnt(
            name=g.bass.get_next_instruction_name(),
            ins=[*_in_ap, g.lower_ap(idxs_ap), g.lower_val_access(g.to_reg(num_idxs))],
            outs=[g.lower_ap(out_ap)],
            transpose=False, num_idxs=num_idxs, elem_size=elem_size,
            stride_bytes_256=stride_bytes // 256, gen_mode=0,
            single_packet=single_packet, queue_num=queue_num,
            sbuf_tokens_per_rank=0, sbuf_free_dim_per_rank=0,
            sbuf_free_dim_pad_per_rank=0, sbuf_byte_offset=0,
        ))

def build_program(meta):
    import concourse.bass as bass
    import concourse.bacc as bacc
    import concourse.tile as tile
    import concourse.mybir as mybir
    f32, i16, bf16 = mybir.dt.float32, mybir.dt.int16, mybir.dt.bfloat16

    n, n_cores = meta["n"], meta["n_cores"]
    nloc, nw, tot = meta["nloc"], meta["nw"], meta["tot"]
    cnt, cell_off = meta["cnt"], meta["cell_off"]
    calls = plan_calls(cnt, cell_off)
    # chunk -> (call index, position in call); calls are contiguous id runs
    chunk_call = np.zeros(tot, np.int64)
    chunk_cpos = np.zeros(tot, np.int64)
    for ci, (ids, r) in enumerate(calls):
        chunk_call[ids] = ci
        chunk_cpos[ids] = np.arange(len(ids))
    call_off = np.cumsum([0] + [len(ids) for ids, _ in calls])  # chunk cols

    nc = bacc.Bacc("TRN2", target_bir_lowering=False, debug=False,
                   num_devices=n_cores, num_swdge_queues=NB)

    t_in = {}
    t_in["xT"] = nc.dram_tensor("xT", [P, nloc], f32, kind="ExternalInput")
    t_in["dinv_w"] = nc.dram_tensor("dinv_w", [P, nw], f32, kind="ExternalInput")
    t_in["W1p"] = nc.dram_tensor("W1p", [P, 16], f32, kind="ExternalInput")
    t_in["W2p"] = nc.dram_tensor("W2p", [64, 32], f32, kind="ExternalInput")
    t_in["W3p"] = nc.dram_tensor("W3p", [64, 64], f32, kind="ExternalInput")
    for li in range(3):
        t_in[f"brep{li}"] = nc.dram_tensor(f"brep{li}", [P, 64], f32,
                                           kind="ExternalInput")
    t_in["iota128"] = nc.dram_tensor("iota128", [P, P], bf16, kind="ExternalInput")
    t_in["ident"] = nc.dram_tensor("ident", [P, P], f32, kind="ExternalInput")
    t_in["z16"] = nc.dram_tensor("z16", [P, 16], bf16, kind="ExternalInput")
    t_in["gidx"] = nc.dram_tensor("gidx", [P, tot * P // 16], i16,
                                  kind="ExternalInput")
    t_in["gseg"] = nc.dram_tensor("gseg", [P, tot], bf16, kind="ExternalInput")
    pooled = nc.dram_tensor("pooled", [P, 64], f32, kind="ExternalOutput")

    tables = [nc.dram_tensor(f"hws{li}", [n, 128], bf16, addr_space="Shared")
              for li in range(3)]
    bounces = [nc.dram_tensor(f"bounce{li}", [nloc, 128], bf16)
               for li in range(3)]

    with tile.TileContext(nc) as tc:
        with (
            tc.tile_pool(name="const", bufs=1) as cpool,
            tc.tile_pool(name="gt", bufs=8) as gt_pool,
            tc.tile_pool(name="sel", bufs=8) as sel_pool,
            tc.tile_pool(name="stage", bufs=2) as st_pool,
            tc.tile_pool(name="post", bufs=2) as post_pool,
            tc.tile_pool(name="hw", bufs=4) as hw_pool,
            tc.tile_pool(name="acc", bufs=1) as acc_pool,
            tc.tile_pool(name="pagg", bufs=2, space="PSUM") as pagg,
            tc.tile_pool(name="pmm", bufs=3, space="PSUM") as pmm,
        ):
            W1t = cpool.tile([P, 16], f32)
            nc.sync.dma_start(W1t[:], t_in["W1p"].ap())
            W2t = cpool.tile([64, 32], f32)
            nc.sync.dma_start(W2t[:], t_in["W2p"].ap())
            W3t = cpool.tile([64, 64], f32)
            nc.sync.dma_start(W3t[:], t_in["W3p"].ap())
            breps = []
            for li in range(3):
                bt = cpool.tile([P, 64], f32, tag=f"brep{li}")
                nc.sync.dma_start(bt[:], t_in[f"brep{li}"].ap())
                breps.append(bt)
            iota = cpool.tile([P, P], bf16)
            nc.sync.dma_start(iota[:], t_in["iota128"].ap())
            ident = cpool.tile([P, P], f32)
            nc.sync.dma_start(ident[:], t_in["ident"].ap())
            z16 = cpool.tile([P, 16], bf16)
            nc.sync.dma_start(z16[:], t_in["z16"].ap())
            dinv_w = cpool.tile([P, nw], f32)
            nc.sync.dma_start(dinv_w[:], t_in["dinv_w"].ap())
            gidx = cpool.tile([P, tot * P // 16], i16)
            nc.sync.dma_start(gidx[:], t_in["gidx"].ap())
            gseg = cpool.tile([P, tot], bf16)
            nc.sync.dma_start(gseg[:], t_in["gseg"].ap())

            # ---- phase A: table0 rows = dinv * (x @ W1), bf16, then AG
            with tc.tile_pool(name="xa", bufs=3) as xa_pool:
                for t in range(nw):
                    m = min(P, nloc - t * P)
                    xT = xa_pool.tile([P, P], f32, tag="xt")
                    nc.sync.dma_start(xT[:, :m], t_in["xT"].ap()[:, t * P:t * P + m])
                    ps = pmm.tile([P, 64], f32, space="PSUM", tag="mm")
                    nc.tensor.matmul(ps[:m, :16], xT[:, :m],
                                     W1t[:], start=True, stop=True)
                    hw = hw_pool.tile([P, 32], bf16)
                    nc.vector.tensor_scalar_mul(hw[:m, :16], ps[:m, :16],
                                                dinv_w[:m, t:t + 1])
                    nc.vector.tensor_copy(out=hw[:m, 16:32], in_=z16[:m])
                    nc.sync.dma_start(bounces[0].ap()[t * P:t * P + m, :32],
                                      hw[:m, :])
            nc.gpsimd.collective_compute(
                "AllGather", mybir.AluOpType.bypass,
                replica_groups=[list(range(n_cores))],
                ins=[bounces[0].ap()], outs=[tables[0].ap()])

            acc = acc_pool.tile([P, 64], f32)

            for li in range(3):
                F, GEl = FS[li], GE[li]
                wps = BANKC // F
                nst = (nw + wps - 1) // wps
                table = tables[li]
                gt_tiles = {}      # call idx -> tile
                emitted = set()

                def emit_call(ci):
                    ids, r = calls[ci]
                    ncc = len(ids)
                    it_ap = gidx[:, call_off[ci] * 8:(call_off[ci] + ncc) * 8]
                    gt = gt_pool.tile([P, GCALL * GEl], bf16, tag=f"gt{li}")
                    dma_gather_raw(
                        nc, gt[:, :ncc * GEl].rearrange("p (g f) -> p g f",
                                                        f=GEl),
                        table.ap()[r * meta["bsz"]:(r + 1) * meta["bsz"], :GEl],
                        it_ap, ncc * P, GEl, 128, r)
                    gt_tiles[ci] = gt
                    emitted.add(ci)

                for st in range(nst):
                    wlo = st * wps
                    wcnt = min(wps, nw - wlo)
                    ps_st = pagg.tile([P, BANKC], f32, space="PSUM", tag="agg")
                    # gathers covering this staging's chunks, emitted
                    # round-robin across buckets so window-major consumption
                    # can retire buffers (8-buf pool) without WAR cycles
                    need = set()
                    for w in range(wlo, wlo + wcnt):
                        for r in range(NB):
                            for k in range(cnt[w, r]):
                                need.add(int(chunk_call[cell_off[w, r] + k]))
                    by_r = {}
                    for ci in sorted(need):
                        if ci not in emitted:
                            by_r.setdefault(calls[ci][1], []).append(ci)
                    maxlen = max((len(v) for v in by_r.values()), default=0)
                    for k in range(maxlen):
                        for r in sorted(by_r):
                            if k < len(by_r[r]):
                                emit_call(by_r[r][k])
                    # matmuls, window-major
                    for j in range(wcnt):
                        w = wlo + j
                        ids = [int(cell_off[w, r] + k)
                               for r in range(NB) for k in range(cnt[w, r])]
                        # batched sel build for this window's chunks
                        nk = len(ids)
                        sel = sel_pool.tile([P, nk * P], bf16, tag="sel")
                        # chunk ids are contiguous per (w,r) cell; cells of
                        # one window are contiguous in (w,r,k) order
                        g0 = ids[0]
                        nc.vector.tensor_tensor(
                            out=sel[:].rearrange("p (c s) -> p c s", s=P),
                            in0=gseg[:, g0:g0 + nk]
                                .rearrange("p (c o) -> p c o", o=1)
                                .to_broadcast([P, nk, P]),
                            in1=iota[:].rearrange("p (o s) -> p o s", o=1)
                                .to_broadcast([P, nk, P]),
                            op=mybir.AluOpType.is_equal)
                        for q, g in enumerate(ids):
                            ci = int(chunk_call[g])
                            cp = int(chunk_cpos[g])
                            gt = gt_tiles[ci]
                            nc.tensor.matmul(
                                ps_st[:, (j * F):(j * F) + F],
                                sel[:, q * P:(q + 1) * P],
                                gt[:, cp * GEl:cp * GEl + F],
                                start=(q == 0), stop=(q == nk - 1))
                    # drain + post (self-loop term: PSUM sum + own bounce row,
                    # then * dinv[dst])
                    stg = st_pool.tile([P, BANKC], f32, tag="stg")
                    nc.scalar.copy(out=stg[:, :wcnt * F], in_=ps_st[:, :wcnt * F])
                    slt = st_pool.tile([P, BANKC], bf16, tag="slt")
                    for j in range(wcnt):
                        w = wlo + j
                        m = min(P, nloc - w * P)
                        if m > 0:
                            nc.sync.dma_start(
                                slt[:m, j * F:j * F + F],
                                bounces[li].ap()[w * P:w * P + m, :F])
                    nc.vector.tensor_tensor(
                        out=stg[:, :wcnt * F], in0=stg[:, :wcnt * F],
                        in1=slt[:, :wcnt * F], op=mybir.AluOpType.add)
                    v = post_pool.tile([P, BANKC], f32, tag="v")
                    nc.vector.tensor_tensor(
                        out=v[:, :wcnt * F].rearrange("p (j f) -> p j f", f=F),
                        in0=stg[:, :wcnt * F].rearrange("p (j f) -> p j f", f=F),
                        in1=dinv_w[:, wlo:wlo + wcnt]
                            .rearrange("p (j o) -> p j o", o=1)
                            .to_broadcast([P, wcnt, F]),
                        op=mybir.AluOpType.mult)
                    nc.vector.tensor_tensor(
                        out=v[:, :wcnt * F].rearrange("p (j f) -> p j f", f=F),
                        in0=v[:, :wcnt * F].rearrange("p (j f) -> p j f", f=F),
                        in1=breps[li][:, :F].rearrange("p (o f) -> p o f", o=1)
                            .to_broadcast([P, wcnt, F]),
                        op=mybir.AluOpType.add)
                    h = post_pool.tile([P, BANKC], f32, tag="h")
                    nc.scalar.activation(h[:, :wcnt * F], v[:, :wcnt * F],
                                         mybir.ActivationFunctionType.Tanh)
                    for j in range(wcnt):
                        w = wlo + j
                        m = min(P, nloc - w * P)
                        if m <= 0:
                            continue
                        if li == 2:
                            hs = h[:m, j * F:(j + 1) * F]
                            if w == 0:
                                nc.vector.tensor_copy(out=acc[:m, :], in_=hs)
                            else:
                                nc.vector.tensor_tensor(
                                    out=acc[:m, :], in0=acc[:m, :], in1=hs,
                                    op=mybir.AluOpType.max)
                        else:
                            Fn = FS[li + 1]
                            psT = pmm.tile([P, P], f32, space="PSUM", tag="mmT")
                            nc.tensor.transpose(psT[:F, :],
                                                h[:, j * F:(j + 1) * F],
                                                ident[:])
                            hT = post_pool.tile([64, P], f32, tag="hT")
                            nc.scalar.copy(out=hT[:], in_=psT[:64, :])
                            Wn = W2t if li == 0 else W3t
                            ps2 = pmm.tile([P, 64], f32, space="PSUM", tag="mm")
                            nc.tensor.matmul(ps2[:m, :Fn], hT[:, :m], Wn[:],
                                             start=True, stop=True)
                            hwt = hw_pool.tile([P, 64], bf16, tag="hwb")
                            nc.vector.tensor_scalar_mul(
                                hwt[:m, :Fn], ps2[:m, :Fn], dinv_w[:m, w:w + 1])
                            nc.sync.dma_start(
                                bounces[li + 1].ap()[w * P:w * P + m, :Fn],
                                hwt[:m, :Fn])
                if li < 2:
                    nc.gpsimd.collective_compute(
                        "AllGather", mybir.AluOpType.bypass,
                        replica_groups=[list(range(n_cores))],
                        ins=[bounces[li + 1].ap()], outs=[tables[li + 1].ap()])
            nc.sync.dma_start(pooled.ap(), acc[:])
    nc.compile()
    return nc


_CACHE = {}


def kernel(x, W1, b1, W2, b2, W3, b3, Wl, bl, edge_index):
    x = np.asarray(x); edge_index = np.asarray(edge_index)
    ins, meta, _ = build_plan(x, np.asarray(W1), np.asarray(b1), np.asarray(W2),
                              np.asarray(b2), np.asarray(W3), np.asarray(b3),
                              edge_index, n_cores=8)
    key = (x.shape, edge_index.shape, meta["tot"])
    if key not in _CACHE:
        _CACHE[key] = build_program(meta)
    nc = _CACHE[key]
    from concourse.bass_utils import run_bass_kernel_spmd
    res = run_bass_kernel_spmd(nc, ins, core_ids=list(range(8)))
    pool = np.stack([res.results[c]["pooled"] for c in range(8)])
    pooled = pool.max(axis=(0, 1))[:64].astype(np.float32)
    out = pooled[None, :] @ np.asarray(Wl, np.float32) + np.asarray(bl, np.float32)
    return out.astype(np.float32)


# revision 4
# speedup vs baseline: 1.4240x; 1.0665x over previous
"""Distributed 3-layer GCN on 8 Trainium2 NeuronCores — v2.

vs v1: no dma_scatter_add, no DRAM aggregation tables. Edges are sorted by
destination and chunked within 128-dst aligned windows; each chunk's
segment-sum matmul accumulates directly into the window's PSUM slot
(start/stop flags). Tables are bf16 ([n,128] rows, 256B stride), gathers
are bf16, and the 4 src buckets map to the 4 SWDGE queues so Q7
descriptor generation runs on all four core pairs in parallel. The
schedule is padded per (window,bucket) to the max chunk count over cores
so one SPMD program serves all 8 cores; gather indices and segment ids
are SBUF-resident per layer.
"""

import numpy as np

P = 128
GCALL = 32           # max chunks per gather call (4096 idxs)
BANKC = 512          # fp32 cols per PSUM bank
FS = [16, 32, 64]    # layer output widths
GE = [32, 32, 64]    # gathered bf16 elems per layer (>=F; 64B/64B/128B)
NB = 4               # src buckets (int16-addressable tables)


def _wrap16(lin, dtype=np.int16):
    n = len(lin)
    t = np.zeros((P, (n + 15) // 16), dtype)
    idx = np.arange(n)
    for k in range(8):
        t[16 * k + idx % 16, idx // 16] = lin
    return t


def build_plan(x, W1, b1, W2, b2, W3, b3, edge_index, n_cores=8):
    import ml_dtypes
    bf16 = ml_dtypes.bfloat16
    n = x.shape[0]
    nloc = n // n_cores
    bsz = n // NB
    nw = (nloc + P - 1) // P

    # self-loops are handled in the post-pass (local rows, no gather);
    # degree still counts them (PyG default adds them before normalizing)
    src = np.asarray(edge_index[0])
    dst = np.asarray(edge_index[1])
    deg = (np.bincount(dst, minlength=n) + 1).astype(np.float64)
    dinv = (1.0 / np.sqrt(deg)).astype(np.float32)
    core_of = dst // nloc

    # per-core chunk decomposition: (window, bucket) cells, <=128 edges/chunk
    percore = []
    for c in range(n_cores):
        mc = core_of == c
        s_c = src[mc]
        d_c = (dst[mc] - c * nloc).astype(np.int64)
        r_c = s_c // bsz
        srel = (s_c - r_c * bsz).astype(np.int32)
        w_c = d_c // P
        order = np.lexsort((d_c, r_c, w_c))
        percore.append((srel[order], d_c[order], r_c[order], w_c[order]))

    # uniform chunk counts per (window, bucket): max over cores
    cnt = np.zeros((nw, NB), np.int64)
    for (s_o, d_o, r_o, w_o) in percore:
        e = np.zeros((nw, NB), np.int64)
        np.add.at(e, (w_o, r_o), 1)
        cnt = np.maximum(cnt, (e + P - 1) // P)
    cnt = np.maximum(cnt, 1)          # every cell gets >=1 chunk slot
    tot = int(cnt.sum())              # chunks per core per layer (uniform)

    # global chunk id in (w, r, k) order
    cell_off = np.zeros((nw, NB), np.int64)
    cell_off.flat[1:] = np.cumsum(cnt.flat)[:-1]

    meta = dict(n=n, n_cores=n_cores, nloc=nloc, bsz=bsz, nw=nw,
                cnt=cnt, cell_off=cell_off, tot=tot)

    ins = []
    for c in range(n_cores):
        s_o, d_o, r_o, w_o = percore[c]
        ne = len(s_o)
        # position within (w, r) cell
        grp = w_o * NB + r_o
        newg = np.empty(ne, bool)
        newg[0] = True
        newg[1:] = grp[1:] != grp[:-1]
        gstarts = np.flatnonzero(newg)
        pos = np.arange(ne) - np.repeat(gstarts, np.diff(np.append(gstarts, ne)))
        chunk = cell_off[w_o, r_o] + pos // P
        slot = pos % P

        srcv = np.zeros((tot, P), np.int16)
        segv = np.full((tot, P), 255.0, np.float32)
        srcv[chunk, slot] = s_o.astype(np.int16)
        segv[chunk, slot] = (d_o % P).astype(np.float32)

        d = {}
        d["xT"] = np.ascontiguousarray(
            x[c * nloc:(c + 1) * nloc].T).astype(np.float32)
        dv = np.zeros(nw * P, np.float32)
        dv[:nloc] = dinv[c * nloc:(c + 1) * nloc]
        d["dinv_w"] = np.ascontiguousarray(dv.reshape(nw, P).T)
        d["W1p"] = np.asarray(W1, np.float32)
        W2p = np.zeros((64, 32), np.float32); W2p[:16] = W2
        W3p = np.zeros((64, 64), np.float32); W3p[:32] = W3
        d["W2p"], d["W3p"] = W2p, W3p
        for li, (b, F) in enumerate(zip((b1, b2, b3), FS)):
            br = np.zeros((P, 64), np.float32); br[:, :F] = np.asarray(b)[None, :]
            d[f"brep{li}"] = br
        d["iota128"] = np.tile(np.arange(P, dtype=np.float32), (P, 1)).astype(bf16)
        d["ident"] = np.eye(P, dtype=np.float32)
        d["z16"] = np.zeros((P, 16), bf16)
        d["gseg"] = np.ascontiguousarray(segv.T).astype(bf16)   # [P, tot]

        # gather idx flat layout in CALL order (bucket-major runs of <=32
        # chunks within each bucket, windows ascending)
        calls = plan_calls(cnt, cell_off)
        gidx = np.zeros((P, tot * P // 16), np.int16)
        off = 0
        for (chunks, r) in calls:
            lin = srcv[chunks].reshape(-1)
            w = _wrap16(lin)
            gidx[:, off:off + w.shape[1]] = w
            off += w.shape[1]
        d["gidx"] = gidx
        ins.append(d)
    return ins, meta, dinv


def plan_calls(cnt, cell_off):
    """Call schedule: per bucket, runs of <=GCALL chunks over ascending
    windows. Returns list of (chunk_id_array, bucket)."""
    nw = cnt.shape[0]
    calls = []
    for r in range(NB):
        ids = np.concatenate([cell_off[w, r] + np.arange(cnt[w, r])
                              for w in range(nw)])
        for i in range(0, len(ids), GCALL):
            calls.append((ids[i:i + GCALL], r))
    return calls



def dma_gather_raw(nc, out_ap, in_ap, idxs_ap, num_idxs, elem_size, elem_step,
                   queue_num, single_packet=False):
    """bass dma_gather with the elem%256B assert relaxed (ucode only requires
    the table stride to be a multiple of 256B)."""
    import concourse.mybir as mybir
    g = nc.gpsimd
    g._assert_queue_num(queue_num)
    stride_bytes = elem_step * mybir.dt.size(in_ap.dtype)
    assert stride_bytes % 256 == 0
    _in_ap = g.lower_ap_dma(in_ap, for_custom_bir_dma=True)
    return g.add_instruction(
        mybir.InstDMAGatherAnt(
            name=g.bass.get_next_instruction_name(),
            ins=[*_in_ap, g.lower_ap(idxs_ap), g.lower_val_access(g.to_reg(num_idxs))],
            outs=[g.lower_ap(out_ap)],
            transpose=False, num_idxs=num_idxs, elem_size=elem_size,
            stride_bytes_256=stride_bytes // 256, gen_mode=0,
            single_packet=single_packet, queue_num=queue_num,
            sbuf_tokens_per_rank=0, sbuf_free_dim_per_rank=0,
            sbuf_free_dim_pad_per_rank=0, sbuf_byte_offset=0,
        ))

def build_program(meta):
    import concourse.bass as bass
    import concourse.bacc as bacc
    import concourse.tile as tile
    import concourse.mybir as mybir
    f32, i16, bf16 = mybir.dt.float32, mybir.dt.int16, mybir.dt.bfloat16

    n, n_cores = meta["n"], meta["n_cores"]
    nloc, nw, tot = meta["nloc"], meta["nw"], meta["tot"]
    cnt, cell_off = meta["cnt"], meta["cell_off"]
    calls = plan_calls(cnt, cell_off)
    # chunk -> (call index, position in call); calls are contiguous id runs
    chunk_call = np.zeros(tot, np.int64)
    chunk_cpos = np.zeros(tot, np.int64)
    for ci, (ids, r) in enumerate(calls):
        chunk_call[ids] = ci
        chunk_cpos[ids] = np.arange(len(ids))
    call_off = np.cumsum([0] + [len(ids) for ids, _ in calls])  # chunk cols

    nc = bacc.Bacc("TRN2", target_bir_lowering=False, debug=False,
                   num_devices=n_cores, num_swdge_queues=NB)

    t_in = {}
    t_in["xT"] = nc.dram_tensor("xT", [P, nloc], f32, kind="ExternalInput")
    t_in["dinv_w"] = nc.dram_tensor("dinv_w", [P, nw], f32, kind="ExternalInput")
    t_in["W1p"] = nc.dram_tensor("W1p", [P, 16], f32, kind="ExternalInput")
    t_in["W2p"] = nc.dram_tensor("W2p", [64, 32], f32, kind="ExternalInput")
    t_in["W3p"] = nc.dram_tensor("W3p", [64, 64], f32, kind="ExternalInput")
    for li in range(3):
        t_in[f"brep{li}"] = nc.dram_tensor(f"brep{li}", [P, 64], f32,
                                           kind="ExternalInput")
    t_in["iota128"] = nc.dram_tensor("iota128", [P, P], bf16, kind="ExternalInput")
    t_in["ident"] = nc.dram_tensor("ident", [P, P], f32, kind="ExternalInput")
    t_in["z16"] = nc.dram_tensor("z16", [P, 16], bf16, kind="ExternalInput")
    t_in["gidx"] = nc.dram_tensor("gidx", [P, tot * P // 16], i16,
                                  kind="ExternalInput")
    t_in["gseg"] = nc.dram_tensor("gseg", [P, tot], bf16, kind="ExternalInput")
    pooled = nc.dram_tensor("pooled", [P, 64], f32, kind="ExternalOutput")

    tables = [nc.dram_tensor(f"hws{li}", [n, 128], bf16, addr_space="Shared")
              for li in range(3)]
    bounces = [nc.dram_tensor(f"bounce{li}", [nloc, 128], bf16)
               for li in range(3)]

    with tile.TileContext(nc) as tc:
        with (
            tc.tile_pool(name="const", bufs=1) as cpool,
            tc.tile_pool(name="gt", bufs=10) as gt_pool,
            tc.tile_pool(name="sel", bufs=8) as sel_pool,
            tc.tile_pool(name="stage", bufs=2) as st_pool,
            tc.tile_pool(name="post", bufs=2) as post_pool,
            tc.tile_pool(name="hw", bufs=4) as hw_pool,
            tc.tile_pool(name="acc", bufs=1) as acc_pool,
            tc.tile_pool(name="pagg", bufs=3, space="PSUM") as pagg,
            tc.tile_pool(name="pmm", bufs=2, space="PSUM") as pmm,
        ):
            W1t = cpool.tile([P, 16], f32)
            nc.sync.dma_start(W1t[:], t_in["W1p"].ap())
            W2t = cpool.tile([64, 32], f32)
            nc.sync.dma_start(W2t[:], t_in["W2p"].ap())
            W3t = cpool.tile([64, 64], f32)
            nc.sync.dma_start(W3t[:], t_in["W3p"].ap())
            breps = []
            for li in range(3):
                bt = cpool.tile([P, 64], f32, tag=f"brep{li}")
                nc.sync.dma_start(bt[:], t_in[f"brep{li}"].ap())
                breps.append(bt)
            iota = cpool.tile([P, P], bf16)
            nc.sync.dma_start(iota[:], t_in["iota128"].ap())
            ident = cpool.tile([P, P], f32)
            nc.sync.dma_start(ident[:], t_in["ident"].ap())
            z16 = cpool.tile([P, 16], bf16)
            nc.sync.dma_start(z16[:], t_in["z16"].ap())
            dinv_w = cpool.tile([P, nw], f32)
            nc.sync.dma_start(dinv_w[:], t_in["dinv_w"].ap())
            gidx = cpool.tile([P, tot * P // 16], i16)
            nc.sync.dma_start(gidx[:], t_in["gidx"].ap())
            gseg = cpool.tile([P, tot], bf16)
            nc.sync.dma_start(gseg[:], t_in["gseg"].ap())

            # ---- phase A: table0 rows = dinv * (x @ W1), bf16, then AG
            with tc.tile_pool(name="xa", bufs=3) as xa_pool:
                for t in range(nw):
                    m = min(P, nloc - t * P)
                    xT = xa_pool.tile([P, P], f32, tag="xt")
                    nc.sync.dma_start(xT[:, :m], t_in["xT"].ap()[:, t * P:t * P + m])
                    ps = pmm.tile([P, 64], f32, space="PSUM", tag="mm")
                    nc.tensor.matmul(ps[:m, :16], xT[:, :m],
                                     W1t[:], start=True, stop=True)
                    hw = hw_pool.tile([P, 32], bf16)
                    nc.vector.tensor_scalar_mul(hw[:m, :16], ps[:m, :16],
                                                dinv_w[:m, t:t + 1])
                    nc.vector.tensor_copy(out=hw[:m, 16:32], in_=z16[:m])
                    nc.sync.dma_start(bounces[0].ap()[t * P:t * P + m, :32],
                                      hw[:m, :])
            nc.gpsimd.collective_compute(
                "AllGather", mybir.AluOpType.bypass,
                replica_groups=[list(range(n_cores))],
                ins=[bounces[0].ap()], outs=[tables[0].ap()])

            acc = acc_pool.tile([P, 64], f32)

            for li in range(3):
                F, GEl = FS[li], GE[li]
                wps = BANKC // F
                nst = (nw + wps - 1) // wps
                table = tables[li]
                gt_tiles = {}      # call idx -> tile
                emitted = set()

                def emit_call(ci):
                    ids, r = calls[ci]
                    ncc = len(ids)
                    it_ap = gidx[:, call_off[ci] * 8:(call_off[ci] + ncc) * 8]
                    gt = gt_pool.tile([P, GCALL * GEl], bf16, tag=f"gt{li}")
                    dma_gather_raw(
                        nc, gt[:, :ncc * GEl].rearrange("p (g f) -> p g f",
                                                        f=GEl),
                        table.ap()[r * meta["bsz"]:(r + 1) * meta["bsz"], :GEl],
                        it_ap, ncc * P, GEl, 128, r)
                    gt_tiles[ci] = gt
                    emitted.add(ci)

                for st in range(nst):
                    wlo = st * wps
                    wcnt = min(wps, nw - wlo)
                    ps_st = pagg.tile([P, BANKC], f32, space="PSUM", tag="agg")
                    # gathers covering this staging's chunks, emitted
                    # round-robin across buckets so window-major consumption
                    # can retire buffers (8-buf pool) without WAR cycles
                    need = set()
                    for w in range(wlo, wlo + wcnt):
                        for r in range(NB):
                            for k in range(cnt[w, r]):
                                need.add(int(chunk_call[cell_off[w, r] + k]))
                    by_r = {}
                    for ci in sorted(need):
                        if ci not in emitted:
                            by_r.setdefault(calls[ci][1], []).append(ci)
                    maxlen = max((len(v) for v in by_r.values()), default=0)
                    for k in range(maxlen):
                        for r in sorted(by_r):
                            if k < len(by_r[r]):
                                emit_call(by_r[r][k])
                    # matmuls, window-major
                    for j in range(wcnt):
                        w = wlo + j
                        ids = [int(cell_off[w, r] + k)
                               for r in range(NB) for k in range(cnt[w, r])]
                        # batched sel build for this window's chunks
                        nk = len(ids)
                        sel = sel_pool.tile([P, nk * P], bf16, tag="sel")
                        # chunk ids are contiguous per (w,r) cell; cells of
                        # one window are contiguous in (w,r,k) order
                        g0 = ids[0]
                        nc.vector.tensor_tensor(
                            out=sel[:].rearrange("p (c s) -> p c s", s=P),
                            in0=gseg[:, g0:g0 + nk]
                                .rearrange("p (c o) -> p c o", o=1)
                                .to_broadcast([P, nk, P]),
                            in1=iota[:].rearrange("p (o s) -> p o s", o=1)
                                .to_broadcast([P, nk, P]),
                            op=mybir.AluOpType.is_equal)
                        for q, g in enumerate(ids):
                            ci = int(chunk_call[g])
                            cp = int(chunk_cpos[g])
                            gt = gt_tiles[ci]
                            nc.tensor.matmul(
                                ps_st[:, (j * F):(j * F) + F],
                                sel[:, q * P:(q + 1) * P],
                                gt[:, cp * GEl:cp * GEl + F],
                                start=(q == 0), stop=(q == nk - 1))
                    # drain + post (self-loop term: PSUM sum + own bounce row,
                    # then * dinv[dst])
                    stg = st_pool.tile([P, BANKC], f32, tag="stg")
                    nc.scalar.copy(out=stg[:, :wcnt * F], in_=ps_st[:, :wcnt * F])
                    slt = st_pool.tile([P, BANKC], bf16, tag="slt")
                    for j in range(wcnt):
                        w = wlo + j
                        m = min(P, nloc - w * P)
                        if m > 0:
                            nc.sync.dma_start(
                                slt[:m, j * F:j * F + F],
                                bounces[li].ap()[w * P:w * P + m, :F])
                    nc.vector.tensor_tensor(
                        out=stg[:, :wcnt * F], in0=stg[:, :wcnt * F],
                        in1=slt[:, :wcnt * F], op=mybir.AluOpType.add)
                    v = post_pool.tile([P, BANKC], f32, tag="v")
                    nc.vector.tensor_tensor(
                        out=v[:, :wcnt * F].rearrange("p (j f) -> p j f", f=F),
                        in0=stg[:, :wcnt * F].rearrange("p (j f) -> p j f", f=F),
                        in1=dinv_w[:, wlo:wlo + wcnt]
                            .rearrange("p (j o) -> p j o", o=1)
                            .to_broadcast([P, wcnt, F]),
                        op=mybir.AluOpType.mult)
                    nc.vector.tensor_tensor(
                        out=v[:, :wcnt * F].rearrange("p (j f) -> p j f", f=F),
                        in0=v[:, :wcnt * F].rearrange("p (j f) -> p j f", f=F),
                        in1=breps[li][:, :F].rearrange("p (o f) -> p o f", o=1)
                            .to_broadcast([P, wcnt, F]),
                        op=mybir.AluOpType.add)
                    h = post_pool.tile([P, BANKC], f32, tag="h")
                    nc.scalar.activation(h[:, :wcnt * F], v[:, :wcnt * F],
                                         mybir.ActivationFunctionType.Tanh)
                    for j in range(wcnt):
                        w = wlo + j
                        m = min(P, nloc - w * P)
                        if m <= 0:
                            continue
                        if li == 2:
                            hs = h[:m, j * F:(j + 1) * F]
                            if w == 0:
                                nc.vector.tensor_copy(out=acc[:m, :], in_=hs)
                            else:
                                nc.vector.tensor_tensor(
                                    out=acc[:m, :], in0=acc[:m, :], in1=hs,
                                    op=mybir.AluOpType.max)
                        else:
                            Fn = FS[li + 1]
                            psT = pmm.tile([P, P], f32, space="PSUM", tag="mmT")
                            nc.tensor.transpose(psT[:F, :],
                                                h[:, j * F:(j + 1) * F],
                                                ident[:])
                            hT = post_pool.tile([64, P], f32, tag="hT")
                            nc.scalar.copy(out=hT[:], in_=psT[:64, :])
                            Wn = W2t if li == 0 else W3t
                            ps2 = pmm.tile([P, 64], f32, space="PSUM", tag="mm")
                            nc.tensor.matmul(ps2[:m, :Fn], hT[:, :m], Wn[:],
                                             start=True, stop=True)
                            hwt = hw_pool.tile([P, 64], bf16, tag="hwb")
                            nc.vector.tensor_scalar_mul(
                                hwt[:m, :Fn], ps2[:m, :Fn], dinv_w[:m, w:w + 1])
                            nc.sync.dma_start(
                                bounces[li + 1].ap()[w * P:w * P + m, :Fn],
                                hwt[:m, :Fn])
                if li < 2:
                    nc.gpsimd.collective_compute(
                        "AllGather", mybir.AluOpType.bypass,
                        replica_groups=[list(range(n_cores))],
                        ins=[bounces[li + 1].ap()], outs=[tables[li + 1].ap()])
            nc.sync.dma_start(pooled.ap(), acc[:])
    nc.compile()
    return nc


_CACHE = {}


def kernel(x, W1, b1, W2, b2, W3, b3, Wl, bl, edge_index):
    x = np.asarray(x); edge_index = np.asarray(edge_index)
    ins, meta, _ = build_plan(x, np.asarray(W1), np.asarray(b1), np.asarray(W2),
                              np.asarray(b2), np.asarray(W3), np.asarray(b3),
                              edge_index, n_cores=8)
    key = (x.shape, edge_index.shape, meta["tot"])
    if key not in _CACHE:
        _CACHE[key] = build_program(meta)
    nc = _CACHE[key]
    from concourse.bass_utils import run_bass_kernel_spmd
    res = run_bass_kernel_spmd(nc, ins, core_ids=list(range(8)))
    pool = np.stack([res.results[c]["pooled"] for c in range(8)])
    pooled = pool.max(axis=(0, 1))[:64].astype(np.float32)
    out = pooled[None, :] @ np.asarray(Wl, np.float32) + np.asarray(bl, np.float32)
    return out.astype(np.float32)
